# revision 11
# baseline (speedup 1.0000x reference)
"""Trainium2 Bass kernel for span-attention pooling.

Problem shapes (hardcoded):
  x: [B=2, T=512, E=1024] f32, W: [1024, 1] f32, b: [1] f32,
  start/end: [S=2048] i32.  Output: [B, S, E] f32.

Math: out[b,s,:] = sum_{t=start[s]}^{end[s]} q[b,t] * x[b,t,:] / sum q[b,t]
with q = exp(relu(x @ W + b)).  (Equivalent to the reference's per-span
softmax over head scores, since spans are contiguous token ranges and
clamped/invalid positions carry zero weight.)

Sharding: 8 cores = 8 groups of 256 spans (sorted by start); each core
handles BOTH batches for its group. A sorted eighth's tokens live in a
window of <= 128 tokens (typ. ~100), so each core loads two [128, E]
x slices (one per batch). Fallback to 256/512-token windows if an
exotic span distribution breaks the window property.

Per core (3 input DMAs, issue time is ~0.7us per dma_start so inputs
are packed):
  - Sync ring: replicated W [128, E] f16, then x[b0] [128, E] f16.
  - Scalar ring: packed [logmask | x[b1]] [128, SQ+E] f16 per chunk.
  - h[t] = sum_e x[t,e] W[e]: one full-width scalar_tensor_tensor per
    batch on Vector (accum_out).
  - rh = relu(h + b) on Scalar ([128,1]).
  - maskq[t,s] = exp(logM[t,s] + rh[t]) in ONE Scalar op (logM is 0
    for valid span positions, -30000 otherwise, so exp gives q or 0).
  - po[s,e] = maskq^T @ x, Z[s] = maskq^T @ 1 on the PE (PE is kept
    busy with warm-up matmuls from body start so the HAM clock gate
    releases before the real matmuls).
  - out = po * (1/Z) in fp16: 6 bank-norms on Vector, 2 on Scalar.
  - 4 row-contiguous 256 KB stores on the Sync ring (last one split
    across both rings).

Outputs travel as fp16 (absmax-rel err ~5e-4 vs the 2e-2 gate) and the
host upcasts to f32.
"""

import numpy as np
import ml_dtypes

import concourse.bass as bass
import concourse.tile as tile
from concourse import bacc, mybir
from concourse import bass_utils

B, T, E = 2, 512, 1024
S, A = 2048, 30
N_CORES = 8
SQ = S // N_CORES  # spans per core (group)

F32 = mybir.dt.float32
F16 = mybir.dt.float16
F8 = mybir.dt.float8e4
I32 = mybir.dt.int32

N_WARM = 6  # N=512 dummy matmuls bridging the HAM clock-gate window
NEG = -448.0  # log-mask "minus infinity" (exp -> 0; max magnitude of fp8e4)


def _build_body(tc, tch, with_bias, out_d, x0_d, x1_d, wt_d, m_d, misc_d):
    nc = tc.nc
    AF = mybir.ActivationFunctionType
    OP = mybir.AluOpType

    with (
        tc.tile_pool(name="main", bufs=1) as mainp,
        tc.tile_pool(name="psum", bufs=1, space="PSUM") as psp,
        tc.tile_pool(name="scr", bufs=1) as scrp,
    ):
        # Scalar ring: W row (tiny) first; Sync ring: batch-0 x chunks.
        wt = mainp.tile([1, E], F16)
        nc.scalar.dma_start(wt[:], wt_d[:])
        x0s = []
        for i in range(tch):
            xt = mainp.tile([128, E], F16, name=f"x0{i}", tag=f"x0{i}")
            nc.sync.dma_start(xt[:], x0_d[128 * i : 128 * (i + 1), :])
            x0s.append(xt)
        # Scalar ring: fp8 log-mask, then batch-1 x chunks.
        if with_bias:
            misc = mainp.tile([128, 2], F16)
            nc.scalar.dma_start(misc[:], misc_d[:])
            bb = misc[:, 0:2].bitcast(F32)
        else:
            bb = 0.0
        m16s = []
        for i in range(tch):
            m8 = mainp.tile([128, SQ], F8, name=f"m{i}", tag=f"m{i}")
            nc.scalar.dma_start(m8[:], m_d[128 * i : 128 * (i + 1), :])
            m16s.append(m8)
        x1s = []
        for i in range(tch):
            xt = mainp.tile([128, E], F16, name=f"x1{i}", tag=f"x1{i}")
            nc.scalar.dma_start(xt[:], x1_d[128 * i : 128 * (i + 1), :])
            x1s.append(xt)
        xts = [x0s, x1s]

        # ones: Z-matmul rhs + W-broadcast lhsT + warm-up operand.
        ones16 = mainp.tile([128, 512], F16)
        nc.vector.memset(ones16[:], 1.0)

        # Broadcast W to all partitions: ones[1,128]^T @ w[1,512] per
        # half; PSUM->SBUF copies split across Vector and Scalar.
        wb16 = mainp.tile([128, E], F16)
        warm = psp.tile([128, 512], F32, name="warm", tag="p7")
        warm2 = psp.tile([128, 512], F32, name="warm2", tag="p2")
        for half in range(2):
            ps = warm if half == 0 else warm2
            nc.tensor.matmul(
                ps[:], ones16[0:1, 0:128], wt[0:1, 512 * half : 512 * (half + 1)],
                start=True, stop=True,
            )
            dst = wb16[:, 512 * half : 512 * (half + 1)]
            if half == 0:
                nc.vector.tensor_scalar_mul(dst, ps[:], 1.0)
            else:
                nc.scalar.mul(dst, ps[:], 1.0)

        # PE warm-up: HAM clock gate releases after ~3.4us of sustained
        # activity; dummy matmuls cover the gap until maskq is ready.
        for _ in range(N_WARM):
            nc.tensor.matmul(warm[:], ones16[:, 0:128], ones16[:], start=True, stop=True)

        # Head scores + maskq, per (batch, token-chunk).
        hc = mainp.tile([128, 2 * tch], F32)
        hb = mainp.tile([128, 2 * tch], F32)
        rh = mainp.tile([128, 2 * tch], F32)
        scrA = scrp.tile([128, E], F16)
        mqs = [[], []]
        for b in range(2):
            for i in range(tch):
                c = b * tch + i
                # h = sum_e x[t,e] W[e]
                nc.vector.scalar_tensor_tensor(
                    scrA[:], xts[b][i][:], 1.0, wb16[:],
                    op0=OP.mult, op1=OP.mult, accum_out=hc[:, c : c + 1],
                )
                with tc.high_priority():
                    if with_bias:
                        nc.vector.tensor_scalar_add(
                            hb[:, c : c + 1], hc[:, c : c + 1], bb
                        )
                        hcol = hb[:, c : c + 1]
                    else:
                        hcol = hc[:, c : c + 1]
                    nc.scalar.activation(rh[:, c : c + 1], hcol, AF.Relu)
                    # maskq[t,s] = exp(logM[t,s] + rh[t]) in one op.
                    mq = mainp.tile([128, SQ], F16, name=f"mq{b}{i}", tag=f"mq{b}{i}")
                    nc.scalar.activation(
                        mq[:], m16s[i][:], AF.Exp, bias=rh[:, c : c + 1]
                    )
                mqs[b].append(mq)

        # Matmuls. 8 PSUM banks: poA/poB for (b0,j0),(b0,j1) and poA for
        # (b1,j0),(b1,j1) get their own banks; Z columns share one bank;
        # poB(b1,j0) reuses the warm-up bank and poB(b1,j1) reuses
        # poA(b0,j0)'s (freed by the first norm).
        SCH = SQ // 128  # span chunks of 128 partitions (2)
        po = {}
        tags = {
            (0, 0, 0): "p0", (0, 0, 1): "p2",
            (0, 1, 0): "p1", (0, 1, 1): "p3",
            (1, 0, 0): "p4", (1, 0, 1): "p7",
            (1, 1, 0): "p5", (1, 1, 1): "p0",
        }
        for key, tg in tags.items():
            po[key] = psp.tile([128, 512], F32, name=f"po{key}", tag=tg)
        zall = psp.tile([128, 64 * 2 * SCH], F32, name="zall", tag="p6")

        def mm_group(b, j, skip_B=False):
            for i in range(tch):
                st_, sp_ = (i == 0), (i == tch - 1)
                lhsT = mqs[b][i][:, 128 * j : 128 * (j + 1)]
                zc = 64 * (2 * b + j)
                nc.tensor.matmul(
                    zall[:, zc : zc + 64], lhsT, ones16[:, 0:64], start=st_, stop=sp_
                )
                nc.tensor.matmul(
                    po[(b, j, 0)][:], lhsT, xts[b][i][:, 0:512], start=st_, stop=sp_
                )
                if not skip_B:
                    nc.tensor.matmul(
                        po[(b, j, 1)][:], lhsT, xts[b][i][:, 512:1024],
                        start=st_, stop=sp_,
                    )

        def mm_tail(b, j):
            for i in range(tch):
                nc.tensor.matmul(
                    po[(b, j, 1)][:], mqs[b][i][:, 128 * j : 128 * (j + 1)],
                    xts[b][i][:, 512:1024],
                    start=(i == 0), stop=(i == tch - 1),
                )



        # Normalize: obs = po * (1/Z) in fp16. Vector takes 6 of the 8
        # bank-halves, Scalar 2; recips batched per batch.
        obs = {}
        for b in range(2):
            for j in range(SCH):
                obs[(b, j)] = scrp.tile(
                    [128, E], F16, name=f"ob{b}{j}", tag=f"ob{b}{j}"
                )
        rz = scrp.tile([128, 2 * SCH], F32)

        def recip(b, j):
            k = 2 * b + j
            nc.vector.reciprocal(rz[:, k : k + 1], zall[:, 64 * k : 64 * k + 1])

        def norm(b, j, half, eng):
            ob = obs[(b, j)]
            p = po[(b, j, half)]
            r = rz[:, 2 * b + j : 2 * b + j + 1]
            lo = 512 * half
            if eng == "v":
                nc.vector.tensor_scalar_mul(ob[:, lo : lo + 512], p[:], r)
            else:
                nc.scalar.mul(ob[:, lo : lo + 512], p[:], r)

        def store(b, j):
            r0 = 128 * (2 * b + j)
            if (b, j) == (1, 1):
                nc.sync.dma_start(out_d[r0 : r0 + 128, 0:512], obs[(b, j)][:, 0:512])
                nc.scalar.dma_start(
                    out_d[r0 : r0 + 128, 512:1024], obs[(b, j)][:, 512:1024]
                )
            else:
                nc.sync.dma_start(out_d[r0 : r0 + 128, :], obs[(b, j)][:])

        # Interleave matmul groups with recips/norms; free poA(b0,j0)'s
        # bank early (reused by (b1,j1)'s poB).
        mm_group(0, 0)
        with tc.high_priority():
            recip(0, 0)
            norm(0, 0, 0, "v")
        mm_group(0, 1)
        with tc.high_priority():
            recip(0, 1)
        norm(0, 0, 1, "s")
        store(0, 0)
        mm_group(1, 0)
        with tc.high_priority():
            recip(1, 0)
        norm(0, 1, 0, "v")
        norm(0, 1, 1, "s")
        store(0, 1)
        mm_group(1, 1, skip_B=True)
        with tc.high_priority():
            recip(1, 1)
        mm_tail(1, 1)
        norm(1, 0, 0, "v")
        norm(1, 0, 1, "s")
        store(1, 0)
        norm(1, 1, 0, "v")
        norm(1, 1, 1, "s")
        store(1, 1)


def _build(tch, with_bias):
    nc = bacc.Bacc(
        "TRN2",
        target_bir_lowering=False,
        debug=False,
        num_devices=N_CORES,
    )
    x0_d = nc.dram_tensor("x0", [tch * 128, E], F16, kind="ExternalInput").ap()
    x1_d = nc.dram_tensor("x1", [tch * 128, E], F16, kind="ExternalInput").ap()
    wt_d = nc.dram_tensor("wt", [1, E], F16, kind="ExternalInput").ap()
    m_d = nc.dram_tensor("m", [tch * 128, SQ], F8, kind="ExternalInput").ap()
    misc_d = None
    if with_bias:
        misc_d = nc.dram_tensor("misc", [128, 2], F16, kind="ExternalInput").ap()
    out_d = nc.dram_tensor("out", [2 * SQ, E], F16, kind="ExternalOutput").ap()
    with tile.TileContext(nc) as tc:
        _build_body(tc, tch, with_bias, out_d, x0_d, x1_d, wt_d, m_d, misc_d)
    nc.compile()
    return nc


_NC_CACHE = {}


def _get_nc(tch=1, with_bias=False):
    key = (tch, with_bias)
    if key not in _NC_CACHE:
        _NC_CACHE[key] = _build(tch, with_bias)
    return _NC_CACHE[key]


def _make_in_maps(tch, with_bias, x, W, b, start, end, groups, los):
    """groups[g] = span indices for core g; los[g] = first token of g's
    x window. Each group has exactly SQ spans whose tokens fit in
    [los[g], los[g] + 128*tch)."""
    x = np.asarray(x, dtype=np.float32)
    start = np.asarray(start, dtype=np.int32)
    end = np.asarray(end, dtype=np.int32)
    w16 = np.asarray(W, np.float32).reshape(1, E).astype(np.float16)
    nrow = 128 * tch
    in_maps = []
    for core in range(N_CORES):
        idx = groups[core]
        lo = los[core]
        hi = min(lo + nrow, T)
        xw = np.zeros((B, nrow, E), np.float16)
        for bb_idx in range(B):
            xw[bb_idx, : hi - lo] = x[bb_idx, lo:hi].astype(np.float16)
        st2 = (start[idx] - lo)[None, :]
        en2 = (end[idx] - lo)[None, :]
        r = np.arange(nrow, dtype=np.int32)[:, None]
        logm = np.where((r >= st2) & (r <= en2), 0.0, NEG).astype(
            ml_dtypes.float8_e4m3
        )
        im = {
            "x0": np.ascontiguousarray(xw[0]),
            "x1": np.ascontiguousarray(xw[1]),
            "m": np.ascontiguousarray(logm),
            "wt": w16,
        }
        if with_bias:
            misc = np.empty((128, 2), np.float16)
            misc[:] = np.asarray(b, np.float32).reshape(1).view(np.float16)[None, :]
            im["misc"] = misc
        in_maps.append(im)
    return in_maps


def run(x, W, b, start, end, trace=False, trace_cores=None):
    """Run on 8 cores; returns (out[B,S,E] f32, BassKernelResults)."""
    start_np = np.asarray(start, dtype=np.int32)
    end_np = np.asarray(end, dtype=np.int32)
    with_bias = bool(np.any(np.asarray(b, np.float32) != 0.0))

    # Windowed sharding: sort spans by start, split into 8 groups of
    # SQ. Pick the smallest token window (128 * tch) that covers every
    # group; tch=4 (full T) always fits.
    order = np.argsort(start_np, kind="stable")
    groups = [order[g * SQ : (g + 1) * SQ] for g in range(N_CORES)]
    for tch in (1, 2, 4):
        los, ok = [], True
        for idx in groups:
            lo = int(start_np[idx].min())
            hi = int(end_np[idx].max())
            if hi - lo + 1 > 128 * tch:
                ok = False
                break
            los.append(min(lo, T - 1))
        if ok:
            break

    nc = _get_nc(tch, with_bias)
    in_maps = _make_in_maps(tch, with_bias, x, W, b, start, end, groups, los)
    res = bass_utils.run_bass_kernel_spmd(
        nc,
        in_maps,
        core_ids=list(range(N_CORES)),
        trace=trace,
        trace_cores=trace_cores,
    )
    out = np.empty((B, S, E), np.float32)
    for core in range(N_CORES):
        idx = groups[core]
        o = res.results[core]["out"]
        for bb_idx in range(B):
            out[bb_idx, idx] = o[SQ * bb_idx : SQ * (bb_idx + 1)].astype(np.float32)
    return out, res


def kernel(x, W, b, start, end):
    out, _ = run(x, W, b, start, end, trace=False)
    return out
